# revision 7
# baseline (speedup 1.0000x reference)
"""Trainium2 Bass kernel for the ActorCriticCriterion (AIC) masked REINFORCE loss.

Reference computation (per the oracle):
    at_or_after_eos = cumsum(seq == 0, axis=1) > 0
    seq_z  = where(at_or_after_eos, 0, seq)
    mask   = concat([ones(B,1), (seq_z > 0)[:, :-1]], axis=1)
    loss   = sum(-logp * (reward - value) * mask) / sum(mask)

Identity: mask[t] = AND(seq[0..t-1] != 0), mask[0] = 1 — one DVE
tensor_tensor_scan (op0=logical_and) per 128-row group, written shifted;
the leading ones column is a tiny memset.

v2 design (HW-measured op costs):
  - DVE fp32 TT = (N+151)/0.96 ns; fp16 TT = (N/2+151)/0.96 (2x packed
    mode).  Scans are 2 cyc/elem at ANY dtype (no 16-bit speedup).
  - So: cast lp (and val/rew for middle blocks) f32->f16 on the idle ACT
    engine ((224+N)/1.2 ns), then run sub/mult on DVE in fp16.  fp16
    rel-err on the final loss measured ~3e-4 (gate 2e-2).
  - seq stays int32 (scan gets no dtype speedup); mask scan emits f16.
  - Blocks of row-groups A=[1,2,2,2,1]: small first block so the first
    scan starts early, small last block so the post-DMA tail chain is
    short.  Edge blocks skip the val/rew casts (f32 SUB with f16 out) to
    shorten the dependency chain.
  - All num matmuls accumulate into one [1,512] PSUM bank (den likewise):
    ones[128,1].T @ chunk, start/stop spanning the whole kernel.  Both
    results pack into one [1,1024] SBUF tile -> single output DMA.
  - Per-core DVE busy ~33us, ACT ~20us, both under the ~42us DMA floor
    (16.78 MB/core at ~400 GB/s) -> the kernel is DMA-bound.

Sharding: pure data-parallel over B across 8 cores (1024 rows each);
host sums the 8 [1,1024] outputs and divides.
"""

import os
import numpy as np

B, T = 8192, 1024
NCORES = 8
ROWS = B // NCORES          # rows per core
P = 128                     # SBUF partitions
MM = 512                    # matmul free-dim chunk (one PSUM bank)
LAYOUT = (1, 2, 2, 2, 1)    # row-groups of 128 per block

_CACHE: dict = {}


def _build_program(rows: int):
    """Build the Bass/Tile program for one core processing `rows` rows."""
    from contextlib import ExitStack

    import concourse.bacc as bacc
    import concourse.mybir as mybir
    import concourse.tile as tile

    assert sum(LAYOUT) * P == rows

    f32 = mybir.dt.float32
    f16 = mybir.dt.float16
    i32 = mybir.dt.int32
    Op = mybir.AluOpType

    # Bacc (not raw Bass): its compile pipeline splits multi-sem sync waits
    # into event-semaphore instructions — this walrus build allows at most
    # one wait per engine instruction.
    nc = bacc.Bacc()
    seq = nc.dram_tensor("seq", [rows, T], i32, kind="ExternalInput")
    lp = nc.dram_tensor("lp", [rows, T], f32, kind="ExternalInput")
    val = nc.dram_tensor("val", [rows, T], f32, kind="ExternalInput")
    rew = nc.dram_tensor("rew", [rows, T], f32, kind="ExternalInput")
    out = nc.dram_tensor("out", [1, 2 * MM], f32, kind="ExternalOutput")

    subs = []
    r = 0
    for na in LAYOUT:
        subs.append((r, na))
        r += P * na
    assert r == rows
    nblk = len(subs)

    def dram_sub(t, r0, na):
        # rows [r0, r0 + na*P) as [p, a, t] with row = r0 + p*na + a:
        # partition p's a-groups are CONSECUTIVE DRAM rows, so each
        # partition line is one na*4KB contiguous run (fatter DMA
        # descriptors than the (a p) mapping's 4KB runs).
        return t[r0:r0 + na * P, :].rearrange("(p a) t -> p a t", a=na)

    light_tail = bool(int(os.environ.get("K_LIGHT_TAIL", "1")))

    with ExitStack() as ctx:
        tc = ctx.enter_context(tile.TileContext(nc))
        if light_tail:
            # Replace Tile's end-of-kernel epilogue (drain + two all-engine
            # EVSEM barriers + 64-sem clear, ~8-9us) with just the final
            # drain. Safe for re-execution: the Bass preamble dma_reset +
            # sem_clear runs at the START of every execution, so leaving
            # semaphores dirty at kernel end is fine.
            import types

            from concourse.vector_clock import ScopedClock

            def _light_drain_and_barrier(self, tick_clock, wait_clock):
                drain_inst = self.nc.sync.drain()
                wait_clock.add_sem_waits(
                    drain_inst.ins,
                    ScopedClock({None: tick_clock.global_clock}))
                popped = self.nc._tile_sem_poison_stack.pop()
                assert popped is self._sem_poison
                # Deliberately do NOT free the tile sems: Bacc's
                # event-semaphore pass allocates from the free pool after
                # this and must not alias sems still used by the kernel.

            tc._drain_and_barrier = types.MethodType(
                _light_drain_and_barrier, tc)

        const_pool = ctx.enter_context(tc.tile_pool(name="const", bufs=1))
        in_pool = ctx.enter_context(tc.tile_pool(name="in", bufs=2))
        scr_pool = ctx.enter_context(tc.tile_pool(name="scr", bufs=2))
        psum_pool = ctx.enter_context(
            tc.tile_pool(name="psum", bufs=1, space="PSUM"))

        ones = const_pool.tile([P, 1], f16)
        nc.vector.memset(ones[:], 1.0)

        num_ps = psum_pool.tile([1, MM], f32)
        den_ps = psum_pool.tile([1, MM], f32)

        for si, (r0, na) in enumerate(subs):
            first, last = si == 0, si == nblk - 1
            edge = first or last
            tg = f"n{na}" + ("e" if edge else "")
            seq_t = in_pool.tile([P, na, T], i32, tag=f"seq{tg}")
            val_t = in_pool.tile([P, na, T], f32, tag=f"val{tg}")
            rew_t = in_pool.tile([P, na, T], f32, tag=f"rew{tg}")
            lp_t = in_pool.tile([P, na, T], f32, tag=f"lp{tg}")
            nc.sync.dma_start(out=seq_t[:], in_=dram_sub(seq, r0, na))
            if last:
                # T-split the last block's val/lp/rew loads so the final
                # chunk-c1 compute chain starts as soon as its own half
                # lands; rew half c1 is the very last transfer and has the
                # shortest chain behind it (sub -> q -> mq -> matmul).
                vd, ld, rd = (dram_sub(x, r0, na) for x in (val, lp, rew))
                nc.sync.dma_start(out=val_t[:, :, 0:MM], in_=vd[:, :, 0:MM])
                nc.sync.dma_start(out=lp_t[:, :, 0:MM], in_=ld[:, :, 0:MM])
                nc.sync.dma_start(out=rew_t[:, :, 0:MM], in_=rd[:, :, 0:MM])
                nc.sync.dma_start(out=val_t[:, :, MM:T], in_=vd[:, :, MM:T])
                nc.sync.dma_start(out=lp_t[:, :, MM:T], in_=ld[:, :, MM:T])
                nc.sync.dma_start(out=rew_t[:, :, MM:T], in_=rd[:, :, MM:T])
            else:
                nc.sync.dma_start(out=val_t[:], in_=dram_sub(val, r0, na))
                nc.sync.dma_start(out=rew_t[:], in_=dram_sub(rew, r0, na))
                nc.sync.dma_start(out=lp_t[:], in_=dram_sub(lp, r0, na))

            # mask[p,a,0] = 1; mask[p,a,t] = AND(seq[p,a,0..t-1] != 0)
            mask = scr_pool.tile([P, na, T], f16, tag=f"mask{tg}", bufs=2)
            nc.vector.memset(mask[:, :, 0:1], 1.0)
            for a in range(na):
                nc.vector.tensor_tensor_scan(
                    out=mask[:, a, 1:T], data0=seq_t[:, a, 0:T - 1],
                    data1=seq_t[:, a, 0:T - 1], initial=1.0,
                    op0=Op.logical_and, op1=Op.bypass)

            # den column sums go to PE as soon as the mask exists.
            for a in range(na):
                for c in range(0, T, MM):
                    nc.tensor.matmul(
                        out=den_ps[:], lhsT=ones[:],
                        rhs=mask[:, a, c:c + MM],
                        start=(si == 0 and a == 0 and c == 0),
                        stop=(last and a == na - 1 and c == T - MM))

            # ACT casts: lp always; val/rew only on middle blocks (edge
            # blocks keep the chain after the last DMA short).
            lp16 = scr_pool.tile([P, na, T], f16, tag=f"lp16{tg}")
            d = scr_pool.tile([P, na, T], f16, tag=f"d{tg}")
            q = scr_pool.tile([P, na, T], f16, tag=f"q{tg}")
            mq = scr_pool.tile([P, na, T], f16, tag=f"mq{tg}")
            if last:
                # chunked pipeline: each T-half flows DMA -> sub -> q ->
                # mq -> matmul independently.
                for c in range(0, T, MM):
                    cs = slice(c, c + MM)
                    nc.scalar.copy(lp16[:, :, cs], lp_t[:, :, cs])
                    nc.vector.tensor_tensor(
                        out=d[:, :, cs], in0=val_t[:, :, cs],
                        in1=rew_t[:, :, cs], op=Op.subtract)
                    nc.vector.tensor_tensor(
                        out=q[:, :, cs], in0=lp16[:, :, cs],
                        in1=d[:, :, cs], op=Op.mult)
                    nc.vector.tensor_tensor(
                        out=mq[:, :, cs], in0=q[:, :, cs],
                        in1=mask[:, :, cs], op=Op.mult)
                    for a in range(na):
                        nc.tensor.matmul(
                            out=num_ps[:], lhsT=ones[:],
                            rhs=mq[:, a, cs],
                            start=False,
                            stop=(a == na - 1 and c == T - MM))
                continue

            nc.scalar.copy(lp16[:], lp_t[:])
            nc.vector.tensor_tensor(out=d[:], in0=val_t[:], in1=rew_t[:],
                                    op=Op.subtract)

            nc.vector.tensor_tensor(out=q[:], in0=lp16[:], in1=d[:],
                                    op=Op.mult)
            nc.vector.tensor_tensor(out=mq[:], in0=q[:], in1=mask[:],
                                    op=Op.mult)

            for a in range(na):
                for c in range(0, T, MM):
                    nc.tensor.matmul(
                        out=num_ps[:], lhsT=ones[:],
                        rhs=mq[:, a, c:c + MM],
                        start=(si == 0 and a == 0 and c == 0),
                        stop=False)

        # PSUM can't be DMA'd directly — bounce through SBUF, den early on
        # scalar, num halves split across both engines, single output DMA.
        out_sb = const_pool.tile([1, 2 * MM], f32)
        nc.scalar.copy(out_sb[:, MM:2 * MM], den_ps[:])
        nc.scalar.copy(out_sb[:, 0:MM // 2], num_ps[:, 0:MM // 2])
        nc.vector.tensor_copy(out_sb[:, MM // 2:MM], num_ps[:, MM // 2:MM])
        nc.sync.dma_start(out=out[:], in_=out_sb[:])

    nc.finalize()
    return nc


def kernel(sample_seq, sample_seqLogprobs, sample_value, sample_reward):
    from concourse.bass_utils import run_bass_kernel_spmd

    seq = np.ascontiguousarray(np.asarray(sample_seq, dtype=np.int32))
    lp = np.ascontiguousarray(np.asarray(sample_seqLogprobs, dtype=np.float32))
    val = np.ascontiguousarray(np.asarray(sample_value, dtype=np.float32))
    rew = np.ascontiguousarray(np.asarray(sample_reward, dtype=np.float32))
    assert seq.shape == (B, T)

    if "nc" not in _CACHE:
        _CACHE["nc"] = _build_program(ROWS)
    nc = _CACHE["nc"]

    in_maps = []
    for c in range(NCORES):
        sl = slice(c * ROWS, (c + 1) * ROWS)
        in_maps.append({
            "seq": seq[sl], "lp": lp[sl], "val": val[sl], "rew": rew[sl],
        })

    trace = bool(int(os.environ.get("K_TRACE", "0")))
    res = run_bass_kernel_spmd(nc, in_maps, core_ids=list(range(NCORES)),
                               trace=trace)
    if trace:
        _CACHE["exec_time_ns"] = res.exec_time_ns
        _CACHE["trace"] = res.instructions_and_trace
    num = 0.0
    den = 0.0
    for r in res.results:
        o = np.asarray(r["out"], dtype=np.float64)
        num += float(o[0, 0:MM].sum())
        den += float(o[0, MM:2 * MM].sum())
    return np.float32(num / den)
